# revision 17
# baseline (speedup 1.0000x reference)
"""ODE-RNN Trainium2 kernel, v2 (gates-major fused recurrence).

out[b, t*8+i, :] = 2-layer GRU (H=1024) over the batch dim (64 steps) of
sequence t (30 sequences), init hiddens from an RK4 ODE trajectory
(8 grid points).  Core i handles the 30 runs with init traj[i].

Per-core structure:
  Phase A: gi1 = x @ wi0.T + bias, gates-major in f32r, written to DRAM
           as [128, 64 steps, 720]; free cols = (tau{r,z}, chunk, run)
           for [0:480], n-gate (chunk, run) for [480:720].
  Loop (64 steps, both layers fused per step): recurrent matmuls are
           gates-major: out tile [128 gates, 30 runs] in PSUM,
           stationary = bf16 weight tile [128 k, 128 gates], moving =
           bf16 state [128 k, 30 runs] (bf16 moving -> 1 cycle/row).
           gi / biases enter PSUM via f32r identity-matmuls (>=256
           wide).  Layer-2's wi1 matmuls accumulate into the same PSUM
           group as wh1 (rz) or a dedicated bank (n), so there is no
           dense gi2 phase and no h1-state saving.  Elementwise is f32
           on ACT/DVE, split into h-chunk halves to pipeline under the
           PE stream.  State kept twice: f32 master + bf16 PE copy.
"""

import numpy as np

try:
    import concourse.bass as bass  # noqa: F401
except ImportError:  # pragma: no cover
    import sys
    sys.path.insert(0, "/opt/trn_rl_repo")
    import concourse.bass as bass  # noqa: F401

import ml_dtypes
import concourse.mybir as mybir
import concourse.tile as tile
from concourse import bacc
from concourse.bass_utils import run_bass_kernel_spmd
from concourse.masks import make_identity

F32 = mybir.dt.float32
F32R = mybir.dt.float32r
BF16 = mybir.dt.bfloat16
AF = mybir.ActivationFunctionType
OP = mybir.AluOpType

H = 1024
KC = 8          # k chunks of 128
NG = 24         # gate tiles (tau*8 + c)
R = 30          # runs per core (exact, no padding)
RP = 32         # padded runs used in phase A only (psum rows >= 256)
S = 64          # steps (batch-as-sequence)
NSEG = 8
SUB = 4
NCORES = 8
T = 30          # sequences


def build_nc(steps=S):
    nc = bacc.Bacc(num_devices=NCORES)

    # per-core x slice: this core's own steps (s = 8*k + core_id)
    xtr = nc.declare_dram_parameter("xtr", [128, KC, 8, 30], BF16, isOutput=False)
    wi0t = nc.declare_dram_parameter("wi0t", [128, KC, 3 * H], BF16, isOutput=False)
    wt0 = nc.declare_dram_parameter("wt0", [128, KC, 3 * H], BF16, isOutput=False)
    wt1 = nc.declare_dram_parameter("wt1", [128, KC, 3 * H], BF16, isOutput=False)
    wt2 = nc.declare_dram_parameter("wt2", [128, KC, 3 * H], BF16, isOutput=False)
    bias1t = nc.declare_dram_parameter("bias1t", [128, NG], F32, isOutput=False)
    b2rz = nc.declare_dram_parameter("b2rz", [128, 480], F32R, isOutput=False)
    bhn1 = nc.declare_dram_parameter("bhn1", [128, 256], F32R, isOutput=False)
    bhn2 = nc.declare_dram_parameter("bhn2", [128, 256], F32R, isOutput=False)
    # step-0 seeds: recurrent contribution of the (constant) init hiddens,
    # precomputed on the host, so step 0 needs no wh0/wh1 matmuls
    g1i0 = nc.declare_dram_parameter("g1i0", [128, 480], F32R, isOutput=False)
    bhn1s0 = nc.declare_dram_parameter("bhn1s0", [128, 256], F32R, isOutput=False)
    b2rz0 = nc.declare_dram_parameter("b2rz0", [128, 480], F32R, isOutput=False)
    bhn2s0 = nc.declare_dram_parameter("bhn2s0", [128, 256], F32R, isOutput=False)
    bi1n = nc.declare_dram_parameter("bi1n", [128, 256], F32R, isOutput=False)
    h1f0 = nc.declare_dram_parameter("h1f0", [128, 240], F32, isOutput=False)
    h2f0 = nc.declare_dram_parameter("h2f0", [128, 240], F32, isOutput=False)
    h1b0 = nc.declare_dram_parameter("h1b0", [128, 240], BF16, isOutput=False)
    h2b0 = nc.declare_dram_parameter("h2b0", [128, 240], BF16, isOutput=False)
    out = nc.declare_dram_parameter("out", [128, S, 240], F32, isOutput=True)

    # collective groups over own-step slices: {0}, {1,2,3}, {4,5,6,7}
    CC_GROUPS = [(0, 1), (1, 4), (4, 8)]
    giA_part = [nc.dram_tensor(f"giA_part{i}", [128, (k1 - k0) * 720], F32R)
                for i, (k0, k1) in enumerate(CC_GROUPS)]
    giAll = [nc.dram_tensor(f"giAll{i}", [NCORES, 128, k1 - k0, 720], F32R)
             for i, (k0, k1) in enumerate(CC_GROUPS)]

    with tile.TileContext(nc) as tc:
        with tc.tile_pool(name="wloop", bufs=1) as wlp:
            # recurrent-weight tiles live across phase A and the loop;
            # their DMAs are emitted after phase A (priority below the
            # gi gather/collective chain)
            w0t = wlp.tile([128, KC, 3 * H], BF16, tag="w0", name="w0t")
            w0 = [w0t[:, kc] for kc in range(KC)]
            w1t = wlp.tile([128, KC, 3 * H], BF16, tag="w1", name="w1t")
            w1 = [w1t[:, kc] for kc in range(KC)]

            identf = wlp.tile([128, 128], F32)
            make_identity(nc, identf)
            identr = wlp.tile([128, 128], F32R)
            nc.vector.tensor_copy(identr, identf)

            # ===== Phase A: this core's 8 gi1 steps (s = 8k + core_id),
            # then 8 pipelined HBM AllGathers deliver the 8-step blocks =====
            with (
                tc.tile_pool(name="wApool", bufs=1) as wApool,
                tc.tile_pool(name="xw_pool", bufs=1) as xwp,
                tc.tile_pool(name="gat_pool", bufs=1) as gatp,
                tc.tile_pool(name="psA", bufs=4, space="PSUM") as psA,
                tc.tile_pool(name="constA", bufs=1) as constA,
            ):
                bias1_sb = constA.tile([128, NG], F32)
                nc.sync.dma_start(out=bias1_sb, in_=bias1t[:])

                xw = xwp.tile([128, KC, 8, 30], BF16, tag="xw", name="xw")
                nc.sync.dma_start(out=xw, in_=xtr[:])
                wiAt = wApool.tile([128, KC, 3 * H], BF16, tag="wiA",
                                   name="wiAt")
                wiA = [wiAt[:, kc] for kc in range(KC)]
                for kc in range(KC):
                    nc.sync.dma_start(out=wiAt[:, kc], in_=wi0t[:, kc])

                gat = gatp.tile([128, 8, 720], F32R, tag="gat", name="gat")
                for pas in range(3):
                    gs = list(range(pas * 8, pas * 8 + 8))
                    pss = [psA.tile([128, 8, 30], F32, tag=f"psA{i}", bufs=1,
                                    name=f"psA_{g}") for i, g in enumerate(gs)]
                    for kc in range(KC):
                        for i, g in enumerate(gs):
                            nc.tensor.matmul(
                                pss[i], wiA[kc][:, g * 128:(g + 1) * 128],
                                xw[:, kc],
                                start=(kc == 0), stop=(kc == KC - 1))
                    for i, g in enumerate(gs):
                        tau, c = g // 8, g % 8
                        off = tau * 240 + c * 30 if tau < 2 else 480 + c * 30
                        nc.vector.tensor_scalar_add(
                            gat[:, :, off:off + 30], pss[i],
                            bias1_sb[:, g:g + 1])
                for i, (k0, k1) in enumerate(CC_GROUPS):
                    nc.sync.dma_start(out=giA_part[i][:],
                                      in_=gat[:, k0:k1, :])
                    nc.gpsimd.collective_compute(
                        kind="AllGather",
                        op=mybir.AluOpType.bypass,
                        replica_groups=[list(range(NCORES))],
                        ins=[giA_part[i][:]],
                        outs=[giAll[i][:]],
                    )

            # weight loads right after phase A's pools free the space
            for kc in range(KC):
                nc.sync.dma_start(out=w0t[:, kc], in_=wt0[:, kc])
            for kc in range(KC):
                nc.sync.dma_start(out=w1t[:, kc], in_=wt1[:, kc])

            # ================= Fused recurrence loop ====================
            with (
                tc.tile_pool(name="wloop2", bufs=1) as wlp2,
                tc.tile_pool(name="constL", bufs=1) as constL,
                tc.tile_pool(name="gi_pool", bufs=2) as gip,
                tc.tile_pool(name="st_pool", bufs=2) as stp,
                tc.tile_pool(name="ew_pool", bufs=2) as ewp,
                tc.tile_pool(name="psL", bufs=2, space="PSUM") as psL,
            ):
                w2t = wlp2.tile([128, KC, 3 * H], BF16, tag="w2", name="w2t")
                w2 = [w2t[:, kc] for kc in range(KC)]

                b2rz_sb = constL.tile([128, 480], F32R)
                nc.sync.dma_start(out=b2rz_sb, in_=b2rz[:])
                bhn1_sb = constL.tile([128, 256], F32R)
                nc.sync.dma_start(out=bhn1_sb, in_=bhn1[:])
                bhn2_sb = constL.tile([128, 256], F32R)
                nc.sync.dma_start(out=bhn2_sb, in_=bhn2[:])
                bi1n_sb = constL.tile([128, 256], F32R)
                nc.sync.dma_start(out=bi1n_sb, in_=bi1n[:])
                g1i0_sb = constL.tile([128, 480], F32R)
                nc.sync.dma_start(out=g1i0_sb, in_=g1i0[:])
                bhn1s0_sb = constL.tile([128, 256], F32R)
                nc.sync.dma_start(out=bhn1s0_sb, in_=bhn1s0[:])
                b2rz0_sb = constL.tile([128, 480], F32R)
                nc.sync.dma_start(out=b2rz0_sb, in_=b2rz0[:])
                bhn2s0_sb = constL.tile([128, 256], F32R)
                nc.sync.dma_start(out=bhn2s0_sb, in_=bhn2s0[:])

                h1f = stp.tile([128, 240], F32, tag="h1f", name="h1f_init")
                nc.sync.dma_start(out=h1f, in_=h1f0[:])
                h2f = stp.tile([128, 240], F32, tag="h2f", name="h2f_init")
                nc.sync.dma_start(out=h2f, in_=h2f0[:])
                h1b = stp.tile([128, 240], BF16, tag="h1b", name="h1b_init")
                nc.sync.dma_start(out=h1b, in_=h1b0[:])
                h2b = stp.tile([128, 240], BF16, tag="h2b", name="h2b_init")
                nc.sync.dma_start(out=h2b, in_=h2b0[:])

                def load_gi(b):
                    t = gip.tile([128, 2, 720], F32R, tag="gw", name=f"gw_{b}")
                    k, r0 = divmod(2 * b, NCORES)
                    i, k0 = (0, 0) if k == 0 else (1, 1) if k < 4 else (2, 4)
                    nc.sync.dma_start(
                        out=t,
                        in_=giAll[i][r0:r0 + 2, :, k - k0]
                        .rearrange("r p c -> p r c"))
                    return t

                gtiles = [load_gi(0)]
                for kc in range(KC):
                    nc.sync.dma_start(out=w2t[:, kc], in_=wt2[:, kc])
                gtiles.append(load_gi(1))

                def rec_mms(dst_rz, dst_n, wts, mov, kcs, stop_rz, stop_n):
                    """Gate matmuls for one layer pass: rz slices into
                    dst_rz (480 wide), n slices into dst_n (240 wide)."""
                    last = kcs[-1]
                    for c in range(8):
                        for tau in range(3):
                            g = tau * 8 + c
                            if tau < 2:
                                dst = dst_rz[:, tau * 240 + c * 30:
                                             tau * 240 + c * 30 + 30]
                                stop_k = last if stop_rz else -1
                            else:
                                dst = dst_n[:, c * 30:c * 30 + 30]
                                stop_k = last if stop_n else -1
                            for kc in kcs:
                                nc.tensor.matmul(
                                    dst,
                                    wts[kc][:, g * 128:(g + 1) * 128],
                                    mov[:, kc * 30:(kc + 1) * 30],
                                    start=False,
                                    stop=(kc == stop_k))

                def elementwise(lab, s, hf, Trz, Tn, hfp, ginA, hf_new, hb_new):
                    """GRU combine for h-chunk half hf (cols hf*120..+120)."""
                    lo = hf * 120
                    t = lambda nm: ewp.tile(
                        [128, 120], F32, tag=f"{nm}{hf}{lab}",
                        name=f"{nm}_{lab}_{s}_{hf}")
                    rz = ewp.tile([128, 2, 120], F32, tag=f"rz{hf}{lab}",
                                  name=f"rz_{lab}_{s}_{hf}")
                    nc.scalar.activation(
                        rz,
                        Trz.rearrange("p (t x) -> p t x", t=2)[:, :, lo:lo + 120],
                        AF.Sigmoid)
                    oz = t("oz")
                    nc.vector.tensor_scalar(oz, rz[:, 1], -1.0, 1.0,
                                            OP.mult, OP.add)
                    bz = t("bz")
                    nc.vector.tensor_mul(bz, rz[:, 1], hfp[:, lo:lo + 120])
                    t1 = t("t1")
                    nc.vector.tensor_mul(t1, rz[:, 0], Tn[:, lo:lo + 120])
                    npre = t("np")
                    nc.vector.tensor_add(npre, t1, ginA)
                    nn = t("nn")
                    nc.scalar.activation(nn, npre, AF.Tanh)
                    aa = t("aa")
                    nc.vector.tensor_mul(aa, nn, oz)
                    nc.vector.tensor_add(hf_new[:, lo:lo + 120], aa, bz)
                    nc.vector.tensor_add(hb_new[:, lo:lo + 120], aa, bz)

                # Software pipeline: iteration i runs L1 matmuls of
                # step i and L2 matmuls of step i-1, so each elementwise
                # chain has a full matmul section of PE work to hide under.
                T1s, T2s = {}, {}
                h1 = {-1: (h1f, h1b)}
                h2 = {-1: (h2f, h2b)}

                for it in range(steps + 1):
                    sL1, sL2 = it, it - 1

                    if sL1 < steps:
                        b, j = divmod(sL1, 2)
                        T1 = psL.tile([128, 480], F32, tag="T1",
                                      name=f"T1_{sL1}")
                        T2 = psL.tile([128, 512], F32, tag="T2",
                                      name=f"T2_{sL1}")
                        T1s[sL1], T2s[sL1] = T1, T2
                        g = gtiles[b]
                        nc.tensor.matmul(T1, identr, g[:, j, 0:480],
                                         start=True, stop=(sL1 == 0 and False))
                        if sL1 == 0:
                            # wh0 @ h1_init precomputed on host
                            nc.tensor.matmul(T1, identr, g1i0_sb,
                                             start=False, stop=True)
                            nc.tensor.matmul(T2[:, 0:256], identr, bhn1s0_sb,
                                             start=True, stop=True)
                        else:
                            nc.tensor.matmul(T2[:, 0:256], identr, bhn1_sb,
                                             start=True, stop=False)
                            rec_mms(T1, T2[:, 0:240], w0, h1[sL1 - 1][1],
                                    list(range(KC)), True, True)

                        # E1(sL1)
                        h1f_new = stp.tile([128, 240], F32, tag="h1f",
                                           name=f"h1f_{sL1}")
                        h1b_new = stp.tile([128, 240], BF16, tag="h1b",
                                           name=f"h1b_{sL1}")
                        for hf in range(2):
                            elementwise(
                                "a", sL1, hf, T1, T2[:, 0:240],
                                h1[sL1 - 1][0],
                                g[:, j, 480 + hf * 120:480 + hf * 120 + 120],
                                h1f_new, h1b_new)
                        h1[sL1] = (h1f_new, h1b_new)

                        if j == 0 and b + 2 < steps // 2:
                            gtiles.append(load_gi(b + 2))

                    if sL2 >= 0:
                        T3 = psL.tile([128, 480], F32, tag="T3",
                                      name=f"T3_{sL2}")
                        T4 = psL.tile([128, 256], F32, tag="T4",
                                      name=f"T4_{sL2}")
                        T2p = T2s.pop(sL2)
                        s0 = sL2 == 0
                        nc.tensor.matmul(T3, identr,
                                         b2rz0_sb if s0 else b2rz_sb,
                                         start=True, stop=False)
                        nc.tensor.matmul(T2p[:, 256:512], identr,
                                         bhn2s0_sb if s0 else bhn2_sb,
                                         start=True, stop=s0)
                        nc.tensor.matmul(T4, identr, bi1n_sb,
                                         start=True, stop=False)
                        # wi1 first (h1b(sL2) is a full iteration old), then
                        # wh1 (its h2b dep is the freshest elementwise)
                        rec_mms(T3, T4[:, 0:240], w1, h1[sL2][1],
                                list(range(KC)), s0, True)
                        if not s0:
                            rec_mms(T3, T2p[:, 256:496], w2, h2[sL2 - 1][1],
                                    list(range(KC)), True, True)

                        # E2(sL2)
                        h2f_new = stp.tile([128, 240], F32, tag="h2f",
                                           name=f"h2f_{sL2}")
                        h2b_new = stp.tile([128, 240], BF16, tag="h2b",
                                           name=f"h2b_{sL2}")
                        for hf in range(2):
                            elementwise(
                                "b", sL2, hf, T3, T2p[:, 256:496],
                                h2[sL2 - 1][0],
                                T4[:, hf * 120:hf * 120 + 120],
                                h2f_new, h2b_new)
                        h2[sL2] = (h2f_new, h2b_new)
                        nc.sync.dma_start(out=out[:, sL2, :], in_=h2f_new)

                        h1.pop(sL2 - 1, None)
                        h2.pop(sL2 - 2, None)
                        T1s.pop(sL2, None)

    nc.finalize()
    return nc


def ode_traj(w1, b1, w2, b2, w3, b3):
    """RK4 trajectory of the ODE, mirroring the reference exactly (fp32)."""
    w1t = w1.T.astype(np.float32)
    w2t = w2.T.astype(np.float32)
    w3t = w3.T.astype(np.float32)

    def f(h):
        a = np.tanh(h @ w1t + b1)
        a = np.tanh(a @ w2t + b2)
        return a @ w3t + b3

    dt = np.float32((1.0 / NSEG) / SUB)
    h = np.zeros((2, H), np.float32)
    traj = []
    for _ in range(NSEG):
        for _ in range(SUB):
            k1 = f(h)
            k2 = f(h + np.float32(0.5) * dt * k1)
            k3 = f(h + np.float32(0.5) * dt * k2)
            k4 = f(h + dt * k3)
            h = h + (dt / np.float32(6.0)) * (k1 + np.float32(2.0) * k2
                                              + np.float32(2.0) * k3 + k4)
        traj.append(h.copy())
    return np.stack(traj)  # (NSEG, 2, H)


def _bc_runs(per_gate, width):
    """[G] gate-vector -> [128, width] broadcast over 30 runs; G = n*128,
    cols laid out (chunk, run) with zero padding to `width`."""
    nchunk = per_gate.size // 128
    a = per_gate.reshape(nchunk, 128)  # [chunk, p]
    o = np.zeros((128, width), np.float32)
    o[:, :nchunk * 30] = np.repeat(
        a.T[:, :, None], 30, axis=2).reshape(128, nchunk * 30)
    return o


def make_in_maps(x, w1, b1, w2, b2, w3, b3, wi0, wh0, bi0, bh0,
                 wi1, wh1, bi1, bh1, cores=NCORES):
    traj = ode_traj(w1, b1, w2, b2, w3, b3)
    bf = ml_dtypes.bfloat16

    # xtr_full[p, kc, s, r] = x[s, r, kc*128+p]; core j later takes its
    # interleaved step slice s = j::8
    xtr_full = np.ascontiguousarray(
        x.reshape(S, T, KC, 128).transpose(3, 2, 0, 1)).astype(bf)

    bias1 = np.concatenate([bi0[:2 * H] + bh0[:2 * H], bi0[2 * H:]])

    shared = {
        "wi0t": np.ascontiguousarray(
            wi0.T.reshape(KC, 128, 3 * H).transpose(1, 0, 2)).astype(bf),
        "wt0": np.ascontiguousarray(
            wh0.T.reshape(KC, 128, 3 * H).transpose(1, 0, 2)).astype(bf),
        "wt1": np.ascontiguousarray(
            wi1.T.reshape(KC, 128, 3 * H).transpose(1, 0, 2)).astype(bf),
        "wt2": np.ascontiguousarray(
            wh1.T.reshape(KC, 128, 3 * H).transpose(1, 0, 2)).astype(bf),
        "bias1t": np.ascontiguousarray(bias1.reshape(NG, 128).T),
        "b2rz": _bc_runs((bi1 + bh1)[:2 * H], 480),
        "bhn1": _bc_runs(bh0[2 * H:], 256),
        "bhn2": _bc_runs(bh1[2 * H:], 256),
        "bi1n": _bc_runs(bi1[2 * H:], 256),
    }
    in_maps = []
    for i in range(cores):
        m = dict(shared)
        m["xtr"] = np.ascontiguousarray(xtr_full[:, :, i::NCORES, :])
        for li, nm in ((0, "h1"), (1, "h2")):
            hf = np.repeat(traj[i, li].reshape(KC, 128).T[:, :, None],
                           30, axis=2).reshape(128, 240)
            m[f"{nm}f0"] = np.ascontiguousarray(hf)
            m[f"{nm}b0"] = np.ascontiguousarray(hf).astype(bf)
        # step-0 recurrent contributions of the constant init hiddens
        gh1i = wh0 @ traj[i, 0]
        gh2i = wh1 @ traj[i, 1]
        m["g1i0"] = _bc_runs(gh1i[:2 * H], 480)
        m["bhn1s0"] = _bc_runs(bh0[2 * H:] + gh1i[2 * H:], 256)
        m["b2rz0"] = m["b2rz"] + _bc_runs(gh2i[:2 * H], 480)
        m["bhn2s0"] = _bc_runs(bh1[2 * H:] + gh2i[2 * H:], 256)
        in_maps.append(m)
    return in_maps


_NC_CACHE = {}


def _get_nc(steps=S):
    if steps not in _NC_CACHE:
        _NC_CACHE[steps] = build_nc(steps)
    return _NC_CACHE[steps]


def run_cores(inputs, steps=S, cores=NCORES, **run_kwargs):
    in_maps = make_in_maps(cores=cores, **inputs)
    nc = _get_nc(steps)
    return run_bass_kernel_spmd(nc, in_maps, core_ids=list(range(cores)),
                                **run_kwargs)


def kernel(x, w1, b1, w2, b2, w3, b3, wi0, wh0, bi0, bh0,
           wi1, wh1, bi1, bh1):
    args = dict(x=x, w1=w1, b1=b1, w2=w2, b2=b2, w3=w3, b3=b3,
                wi0=wi0, wh0=wh0, bi0=bi0, bh0=bh0,
                wi1=wi1, wh1=wh1, bi1=bi1, bh1=bh1)
    args = {k: np.asarray(v, np.float32) for k, v in args.items()}
    res = run_cores(args, steps=S, cores=NCORES)
    B = 64
    full = np.empty((B, T * NCORES, H), np.float32)
    for i in range(NCORES):
        o = np.asarray(res.results[i]["out"], np.float32)
        # out[p, s, c*30+t] -> full[s, t*8+i, c*128+p]
        full[:, i::NCORES, :] = o.reshape(
            128, S, KC, 30).transpose(1, 3, 2, 0).reshape(B, T, H)
    return full


# revision 19
# speedup vs baseline: 1.1836x; 1.1836x over previous
"""ODE-RNN Trainium2 kernel, v2 (gates-major fused recurrence).

out[b, t*8+i, :] = 2-layer GRU (H=1024) over the batch dim (64 steps) of
sequence t (30 sequences), init hiddens from an RK4 ODE trajectory
(8 grid points).  Core i handles the 30 runs with init traj[i].

Per-core structure:
  Phase A: gi1 = x @ wi0.T + bias, gates-major in f32r, written to DRAM
           as [128, 64 steps, 720]; free cols = (tau{r,z}, chunk, run)
           for [0:480], n-gate (chunk, run) for [480:720].
  Loop (64 steps, both layers fused per step): recurrent matmuls are
           gates-major: out tile [128 gates, 30 runs] in PSUM,
           stationary = bf16 weight tile [128 k, 128 gates], moving =
           bf16 state [128 k, 30 runs] (bf16 moving -> 1 cycle/row).
           gi / biases enter PSUM via f32r identity-matmuls (>=256
           wide).  Layer-2's wi1 matmuls accumulate into the same PSUM
           group as wh1 (rz) or a dedicated bank (n), so there is no
           dense gi2 phase and no h1-state saving.  Elementwise is f32
           on ACT/DVE, split into h-chunk halves to pipeline under the
           PE stream.  State kept twice: f32 master + bf16 PE copy.
"""

import numpy as np

try:
    import concourse.bass as bass  # noqa: F401
except ImportError:  # pragma: no cover
    import sys
    sys.path.insert(0, "/opt/trn_rl_repo")
    import concourse.bass as bass  # noqa: F401

import ml_dtypes
import concourse.mybir as mybir
import concourse.tile as tile
from concourse import bacc
from concourse.bass_utils import run_bass_kernel_spmd
from concourse.masks import make_identity

F32 = mybir.dt.float32
F32R = mybir.dt.float32r
BF16 = mybir.dt.bfloat16
AF = mybir.ActivationFunctionType
OP = mybir.AluOpType

H = 1024
KC = 8          # k chunks of 128
NG = 24         # gate tiles (tau*8 + c)
R = 30          # runs per core (exact, no padding)
RP = 32         # padded runs used in phase A only (psum rows >= 256)
S = 64          # steps (batch-as-sequence)
NSEG = 8
SUB = 4
NCORES = 8
T = 30          # sequences


def build_nc(steps=S):
    nc = bacc.Bacc(num_devices=NCORES)

    # per-core x: 24 common steps (0..23) + 5 own steps (24+j, 32+j, ...)
    xtr = nc.declare_dram_parameter("xtr", [128, KC, 32, 30], BF16, isOutput=False)
    wi0t = nc.declare_dram_parameter("wi0t", [128, KC, 3 * H], BF16, isOutput=False)
    wt0 = nc.declare_dram_parameter("wt0", [128, KC, 3 * H], BF16, isOutput=False)
    wt1 = nc.declare_dram_parameter("wt1", [128, KC, 3 * H], BF16, isOutput=False)
    wt2 = nc.declare_dram_parameter("wt2", [128, KC, 3 * H], BF16, isOutput=False)
    bias1t = nc.declare_dram_parameter("bias1t", [128, NG], F32, isOutput=False)
    b2rz = nc.declare_dram_parameter("b2rz", [128, 480], F32R, isOutput=False)
    bhn1 = nc.declare_dram_parameter("bhn1", [128, 256], F32R, isOutput=False)
    bhn2 = nc.declare_dram_parameter("bhn2", [128, 256], F32R, isOutput=False)
    # step-0 seeds: recurrent contribution of the (constant) init hiddens,
    # precomputed on the host, so step 0 needs no wh0/wh1 matmuls
    g1i0 = nc.declare_dram_parameter("g1i0", [128, 480], F32R, isOutput=False)
    bhn1s0 = nc.declare_dram_parameter("bhn1s0", [128, 256], F32R, isOutput=False)
    b2rz0 = nc.declare_dram_parameter("b2rz0", [128, 480], F32R, isOutput=False)
    bhn2s0 = nc.declare_dram_parameter("bhn2s0", [128, 256], F32R, isOutput=False)
    bi1n = nc.declare_dram_parameter("bi1n", [128, 256], F32R, isOutput=False)
    h1f0 = nc.declare_dram_parameter("h1f0", [128, 240], F32, isOutput=False)
    h2f0 = nc.declare_dram_parameter("h2f0", [128, 240], F32, isOutput=False)
    h1b0 = nc.declare_dram_parameter("h1b0", [128, 240], BF16, isOutput=False)
    h2b0 = nc.declare_dram_parameter("h2b0", [128, 240], BF16, isOutput=False)
    out = nc.declare_dram_parameter("out", [128, S, 240], F32, isOutput=True)

    gi1d = nc.dram_tensor("gi1d", [128, 24, 720], F32R)   # steps 0..23
    giA_part = nc.dram_tensor("giA_part", [128, 5 * 720], F32R)
    giAll = nc.dram_tensor("giAll", [NCORES, 128, 5, 720], F32R)

    with tile.TileContext(nc) as tc:
        with tc.tile_pool(name="wloop", bufs=1) as wlp:
            # recurrent-weight tiles live across phase A and the loop;
            # their DMAs are emitted after phase A (priority below the
            # gi gather/collective chain)
            w0t = wlp.tile([128, KC, 3 * H], BF16, tag="w0", name="w0t")
            w0 = [w0t[:, kc] for kc in range(KC)]
            w1t = wlp.tile([128, KC, 3 * H], BF16, tag="w1", name="w1t")
            w1 = [w1t[:, kc] for kc in range(KC)]

            identf = wlp.tile([128, 128], F32)
            make_identity(nc, identf)
            identr = wlp.tile([128, 128], F32R)
            nc.vector.tensor_copy(identr, identf)

            # ===== Phase A: this core's 8 gi1 steps (s = 8k + core_id),
            # then 8 pipelined HBM AllGathers deliver the 8-step blocks =====
            with (
                tc.tile_pool(name="wApool", bufs=1) as wApool,
                tc.tile_pool(name="xw_pool", bufs=1) as xwp,
                tc.tile_pool(name="gat_pool", bufs=1) as gatp,
                tc.tile_pool(name="psA", bufs=4, space="PSUM") as psA,
                tc.tile_pool(name="constA", bufs=1) as constA,
            ):
                bias1_sb = constA.tile([128, NG], F32)
                nc.sync.dma_start(out=bias1_sb, in_=bias1t[:])

                xw = xwp.tile([128, KC, 32, 30], BF16, tag="xw", name="xw")
                nc.sync.dma_start(out=xw, in_=xtr[:])
                wiAt = wApool.tile([128, KC, 3 * H], BF16, tag="wiA",
                                   name="wiAt")
                wiA = [wiAt[:, kc] for kc in range(KC)]
                for kc in range(KC):
                    nc.sync.dma_start(out=wiAt[:, kc], in_=wi0t[:, kc])

                for blk in range(4):
                    gat = gatp.tile([128, 8, 720], F32R, tag="gat",
                                    name=f"gat_{blk}")
                    for pas in range(3):
                        gs = list(range(pas * 8, pas * 8 + 8))
                        pss = [psA.tile([128, 8, 30], F32, tag=f"psA{i}",
                                        bufs=1, name=f"psA_{blk}_{g}")
                               for i, g in enumerate(gs)]
                        for kc in range(KC):
                            for i, g in enumerate(gs):
                                nc.tensor.matmul(
                                    pss[i],
                                    wiA[kc][:, g * 128:(g + 1) * 128],
                                    xw[:, kc, blk * 8:(blk + 1) * 8, :],
                                    start=(kc == 0), stop=(kc == KC - 1))
                        for i, g in enumerate(gs):
                            tau, c = g // 8, g % 8
                            off = tau * 240 + c * 30 if tau < 2 else 480 + c * 30
                            nc.vector.tensor_scalar_add(
                                gat[:, :, off:off + 30], pss[i],
                                bias1_sb[:, g:g + 1])
                    if blk < 3:
                        nc.sync.dma_start(
                            out=gi1d[:, blk * 8:(blk + 1) * 8, :], in_=gat)
                    else:
                        nc.sync.dma_start(out=giA_part[:],
                                          in_=gat[:, 0:5, :])
                        nc.gpsimd.collective_compute(
                            kind="AllGather",
                            op=mybir.AluOpType.bypass,
                            replica_groups=[list(range(NCORES))],
                            ins=[giA_part[:]],
                            outs=[giAll[:]],
                        )

            # weight loads right after phase A's pools free the space
            for kc in range(KC):
                nc.sync.dma_start(out=w0t[:, kc], in_=wt0[:, kc])
            for kc in range(KC):
                nc.sync.dma_start(out=w1t[:, kc], in_=wt1[:, kc])

            # ================= Fused recurrence loop ====================
            with (
                tc.tile_pool(name="wloop2", bufs=1) as wlp2,
                tc.tile_pool(name="constL", bufs=1) as constL,
                tc.tile_pool(name="gi_pool", bufs=2) as gip,
                tc.tile_pool(name="st_pool", bufs=2) as stp,
                tc.tile_pool(name="ew_pool", bufs=2) as ewp,
                tc.tile_pool(name="psL", bufs=2, space="PSUM") as psL,
            ):
                w2t = wlp2.tile([128, KC, 3 * H], BF16, tag="w2", name="w2t")
                w2 = [w2t[:, kc] for kc in range(KC)]

                b2rz_sb = constL.tile([128, 480], F32R)
                nc.sync.dma_start(out=b2rz_sb, in_=b2rz[:])
                bhn1_sb = constL.tile([128, 256], F32R)
                nc.sync.dma_start(out=bhn1_sb, in_=bhn1[:])
                bhn2_sb = constL.tile([128, 256], F32R)
                nc.sync.dma_start(out=bhn2_sb, in_=bhn2[:])
                bi1n_sb = constL.tile([128, 256], F32R)
                nc.sync.dma_start(out=bi1n_sb, in_=bi1n[:])
                g1i0_sb = constL.tile([128, 480], F32R)
                nc.sync.dma_start(out=g1i0_sb, in_=g1i0[:])
                bhn1s0_sb = constL.tile([128, 256], F32R)
                nc.sync.dma_start(out=bhn1s0_sb, in_=bhn1s0[:])
                b2rz0_sb = constL.tile([128, 480], F32R)
                nc.sync.dma_start(out=b2rz0_sb, in_=b2rz0[:])
                bhn2s0_sb = constL.tile([128, 256], F32R)
                nc.sync.dma_start(out=bhn2s0_sb, in_=bhn2s0[:])

                h1f = stp.tile([128, 240], F32, tag="h1f", name="h1f_init")
                nc.sync.dma_start(out=h1f, in_=h1f0[:])
                h2f = stp.tile([128, 240], F32, tag="h2f", name="h2f_init")
                nc.sync.dma_start(out=h2f, in_=h2f0[:])
                h1b = stp.tile([128, 240], BF16, tag="h1b", name="h1b_init")
                nc.sync.dma_start(out=h1b, in_=h1b0[:])
                h2b = stp.tile([128, 240], BF16, tag="h2b", name="h2b_init")
                nc.sync.dma_start(out=h2b, in_=h2b0[:])

                def load_gi(b):
                    t = gip.tile([128, 2, 720], F32R, tag="gw", name=f"gw_{b}")
                    if b < 12:        # steps 0..23: computed locally
                        nc.sync.dma_start(
                            out=t, in_=gi1d[:, b * 2:(b + 1) * 2, :])
                    else:             # steps 24..63: from the AllGather
                        k, r0 = divmod(2 * b, NCORES)
                        nc.sync.dma_start(
                            out=t,
                            in_=giAll[r0:r0 + 2, :, k - 3]
                            .rearrange("r p c -> p r c"))
                    return t

                gtiles = [load_gi(0)]
                for kc in range(KC):
                    nc.sync.dma_start(out=w2t[:, kc], in_=wt2[:, kc])
                gtiles.append(load_gi(1))

                def rec_mms(dst_rz, dst_n, wts, mov, kcs, stop_rz, stop_n):
                    """Gate matmuls for one layer pass: rz slices into
                    dst_rz (480 wide), n slices into dst_n (240 wide)."""
                    last = kcs[-1]
                    for c in range(8):
                        for tau in range(3):
                            g = tau * 8 + c
                            if tau < 2:
                                dst = dst_rz[:, tau * 240 + c * 30:
                                             tau * 240 + c * 30 + 30]
                                stop_k = last if stop_rz else -1
                            else:
                                dst = dst_n[:, c * 30:c * 30 + 30]
                                stop_k = last if stop_n else -1
                            for kc in kcs:
                                nc.tensor.matmul(
                                    dst,
                                    wts[kc][:, g * 128:(g + 1) * 128],
                                    mov[:, kc * 30:(kc + 1) * 30],
                                    start=False,
                                    stop=(kc == stop_k))

                def elementwise(lab, s, hf, Trz, Tn, hfp, ginA, hf_new, hb_new):
                    """GRU combine for h-chunk half hf (cols hf*120..+120)."""
                    lo = hf * 120
                    t = lambda nm: ewp.tile(
                        [128, 120], F32, tag=f"{nm}{hf}{lab}",
                        name=f"{nm}_{lab}_{s}_{hf}")
                    rz = ewp.tile([128, 2, 120], F32, tag=f"rz{hf}{lab}",
                                  name=f"rz_{lab}_{s}_{hf}")
                    nc.scalar.activation(
                        rz,
                        Trz.rearrange("p (t x) -> p t x", t=2)[:, :, lo:lo + 120],
                        AF.Sigmoid)
                    oz = t("oz")
                    nc.vector.tensor_scalar(oz, rz[:, 1], -1.0, 1.0,
                                            OP.mult, OP.add)
                    bz = t("bz")
                    nc.vector.tensor_mul(bz, rz[:, 1], hfp[:, lo:lo + 120])
                    t1 = t("t1")
                    nc.vector.tensor_mul(t1, rz[:, 0], Tn[:, lo:lo + 120])
                    npre = t("np")
                    nc.vector.tensor_add(npre, t1, ginA)
                    nn = t("nn")
                    nc.scalar.activation(nn, npre, AF.Tanh)
                    aa = t("aa")
                    nc.vector.tensor_mul(aa, nn, oz)
                    nc.vector.tensor_add(hf_new[:, lo:lo + 120], aa, bz)
                    nc.vector.tensor_add(hb_new[:, lo:lo + 120], aa, bz)

                # Software pipeline: iteration i runs L1 matmuls of
                # step i and L2 matmuls of step i-1, so each elementwise
                # chain has a full matmul section of PE work to hide under.
                T1s, T2s = {}, {}
                h1 = {-1: (h1f, h1b)}
                h2 = {-1: (h2f, h2b)}

                for it in range(steps + 1):
                    sL1, sL2 = it, it - 1

                    if sL1 < steps:
                        b, j = divmod(sL1, 2)
                        T1 = psL.tile([128, 480], F32, tag="T1",
                                      name=f"T1_{sL1}")
                        T2 = psL.tile([128, 512], F32, tag="T2",
                                      name=f"T2_{sL1}")
                        T1s[sL1], T2s[sL1] = T1, T2
                        g = gtiles[b]
                        nc.tensor.matmul(T1, identr, g[:, j, 0:480],
                                         start=True, stop=(sL1 == 0 and False))
                        if sL1 == 0:
                            # wh0 @ h1_init precomputed on host
                            nc.tensor.matmul(T1, identr, g1i0_sb,
                                             start=False, stop=True)
                            nc.tensor.matmul(T2[:, 0:256], identr, bhn1s0_sb,
                                             start=True, stop=True)
                        else:
                            nc.tensor.matmul(T2[:, 0:256], identr, bhn1_sb,
                                             start=True, stop=False)
                            rec_mms(T1, T2[:, 0:240], w0, h1[sL1 - 1][1],
                                    list(range(KC)), True, True)

                        # E1(sL1)
                        h1f_new = stp.tile([128, 240], F32, tag="h1f",
                                           name=f"h1f_{sL1}")
                        h1b_new = stp.tile([128, 240], BF16, tag="h1b",
                                           name=f"h1b_{sL1}")
                        for hf in range(2):
                            elementwise(
                                "a", sL1, hf, T1, T2[:, 0:240],
                                h1[sL1 - 1][0],
                                g[:, j, 480 + hf * 120:480 + hf * 120 + 120],
                                h1f_new, h1b_new)
                        h1[sL1] = (h1f_new, h1b_new)

                        if j == 0 and b + 2 < steps // 2:
                            gtiles.append(load_gi(b + 2))

                    if sL2 >= 0:
                        T3 = psL.tile([128, 480], F32, tag="T3",
                                      name=f"T3_{sL2}")
                        T4 = psL.tile([128, 256], F32, tag="T4",
                                      name=f"T4_{sL2}")
                        T2p = T2s.pop(sL2)
                        s0 = sL2 == 0
                        nc.tensor.matmul(T3, identr,
                                         b2rz0_sb if s0 else b2rz_sb,
                                         start=True, stop=False)
                        nc.tensor.matmul(T2p[:, 256:512], identr,
                                         bhn2s0_sb if s0 else bhn2_sb,
                                         start=True, stop=s0)
                        nc.tensor.matmul(T4, identr, bi1n_sb,
                                         start=True, stop=False)
                        # wi1 first (h1b(sL2) is a full iteration old), then
                        # wh1 (its h2b dep is the freshest elementwise)
                        rec_mms(T3, T4[:, 0:240], w1, h1[sL2][1],
                                list(range(KC)), s0, True)
                        if not s0:
                            rec_mms(T3, T2p[:, 256:496], w2, h2[sL2 - 1][1],
                                    list(range(KC)), True, True)

                        # E2(sL2)
                        h2f_new = stp.tile([128, 240], F32, tag="h2f",
                                           name=f"h2f_{sL2}")
                        h2b_new = stp.tile([128, 240], BF16, tag="h2b",
                                           name=f"h2b_{sL2}")
                        for hf in range(2):
                            elementwise(
                                "b", sL2, hf, T3, T2p[:, 256:496],
                                h2[sL2 - 1][0],
                                T4[:, hf * 120:hf * 120 + 120],
                                h2f_new, h2b_new)
                        h2[sL2] = (h2f_new, h2b_new)
                        nc.sync.dma_start(out=out[:, sL2, :], in_=h2f_new)

                        h1.pop(sL2 - 1, None)
                        h2.pop(sL2 - 2, None)
                        T1s.pop(sL2, None)

    nc.finalize()
    return nc


def ode_traj(w1, b1, w2, b2, w3, b3):
    """RK4 trajectory of the ODE, mirroring the reference exactly (fp32)."""
    w1t = w1.T.astype(np.float32)
    w2t = w2.T.astype(np.float32)
    w3t = w3.T.astype(np.float32)

    def f(h):
        a = np.tanh(h @ w1t + b1)
        a = np.tanh(a @ w2t + b2)
        return a @ w3t + b3

    dt = np.float32((1.0 / NSEG) / SUB)
    h = np.zeros((2, H), np.float32)
    traj = []
    for _ in range(NSEG):
        for _ in range(SUB):
            k1 = f(h)
            k2 = f(h + np.float32(0.5) * dt * k1)
            k3 = f(h + np.float32(0.5) * dt * k2)
            k4 = f(h + dt * k3)
            h = h + (dt / np.float32(6.0)) * (k1 + np.float32(2.0) * k2
                                              + np.float32(2.0) * k3 + k4)
        traj.append(h.copy())
    return np.stack(traj)  # (NSEG, 2, H)


def _bc_runs(per_gate, width):
    """[G] gate-vector -> [128, width] broadcast over 30 runs; G = n*128,
    cols laid out (chunk, run) with zero padding to `width`."""
    nchunk = per_gate.size // 128
    a = per_gate.reshape(nchunk, 128)  # [chunk, p]
    o = np.zeros((128, width), np.float32)
    o[:, :nchunk * 30] = np.repeat(
        a.T[:, :, None], 30, axis=2).reshape(128, nchunk * 30)
    return o


def make_in_maps(x, w1, b1, w2, b2, w3, b3, wi0, wh0, bi0, bh0,
                 wi1, wh1, bi1, bh1, cores=NCORES):
    traj = ode_traj(w1, b1, w2, b2, w3, b3)
    bf = ml_dtypes.bfloat16

    # xtr_full[p, kc, s, r] = x[s, r, kc*128+p]; core j later takes its
    # interleaved step slice s = j::8
    xtr_full = np.ascontiguousarray(
        x.reshape(S, T, KC, 128).transpose(3, 2, 0, 1)).astype(bf)

    bias1 = np.concatenate([bi0[:2 * H] + bh0[:2 * H], bi0[2 * H:]])

    shared = {
        "wi0t": np.ascontiguousarray(
            wi0.T.reshape(KC, 128, 3 * H).transpose(1, 0, 2)).astype(bf),
        "wt0": np.ascontiguousarray(
            wh0.T.reshape(KC, 128, 3 * H).transpose(1, 0, 2)).astype(bf),
        "wt1": np.ascontiguousarray(
            wi1.T.reshape(KC, 128, 3 * H).transpose(1, 0, 2)).astype(bf),
        "wt2": np.ascontiguousarray(
            wh1.T.reshape(KC, 128, 3 * H).transpose(1, 0, 2)).astype(bf),
        "bias1t": np.ascontiguousarray(bias1.reshape(NG, 128).T),
        "b2rz": _bc_runs((bi1 + bh1)[:2 * H], 480),
        "bhn1": _bc_runs(bh0[2 * H:], 256),
        "bhn2": _bc_runs(bh1[2 * H:], 256),
        "bi1n": _bc_runs(bi1[2 * H:], 256),
    }
    in_maps = []
    for i in range(cores):
        m = dict(shared)
        own = xtr_full[:, :, 24 + i::NCORES, :]       # steps 24+i, 32+i, ...
        xtr_i = np.zeros((128, KC, 32, 30), xtr_full.dtype)
        xtr_i[:, :, :24] = xtr_full[:, :, :24]
        xtr_i[:, :, 24:29] = own
        m["xtr"] = np.ascontiguousarray(xtr_i)
        for li, nm in ((0, "h1"), (1, "h2")):
            hf = np.repeat(traj[i, li].reshape(KC, 128).T[:, :, None],
                           30, axis=2).reshape(128, 240)
            m[f"{nm}f0"] = np.ascontiguousarray(hf)
            m[f"{nm}b0"] = np.ascontiguousarray(hf).astype(bf)
        # step-0 recurrent contributions of the constant init hiddens
        gh1i = wh0 @ traj[i, 0]
        gh2i = wh1 @ traj[i, 1]
        m["g1i0"] = _bc_runs(gh1i[:2 * H], 480)
        m["bhn1s0"] = _bc_runs(bh0[2 * H:] + gh1i[2 * H:], 256)
        m["b2rz0"] = m["b2rz"] + _bc_runs(gh2i[:2 * H], 480)
        m["bhn2s0"] = _bc_runs(bh1[2 * H:] + gh2i[2 * H:], 256)
        in_maps.append(m)
    return in_maps


_NC_CACHE = {}


def _get_nc(steps=S):
    if steps not in _NC_CACHE:
        _NC_CACHE[steps] = build_nc(steps)
    return _NC_CACHE[steps]


def run_cores(inputs, steps=S, cores=NCORES, **run_kwargs):
    in_maps = make_in_maps(cores=cores, **inputs)
    nc = _get_nc(steps)
    return run_bass_kernel_spmd(nc, in_maps, core_ids=list(range(cores)),
                                **run_kwargs)


def kernel(x, w1, b1, w2, b2, w3, b3, wi0, wh0, bi0, bh0,
           wi1, wh1, bi1, bh1):
    args = dict(x=x, w1=w1, b1=b1, w2=w2, b2=b2, w3=w3, b3=b3,
                wi0=wi0, wh0=wh0, bi0=bi0, bh0=bh0,
                wi1=wi1, wh1=wh1, bi1=bi1, bh1=bh1)
    args = {k: np.asarray(v, np.float32) for k, v in args.items()}
    res = run_cores(args, steps=S, cores=NCORES)
    B = 64
    full = np.empty((B, T * NCORES, H), np.float32)
    for i in range(NCORES):
        o = np.asarray(res.results[i]["out"], np.float32)
        # out[p, s, c*30+t] -> full[s, t*8+i, c*128+p]
        full[:, i::NCORES, :] = o.reshape(
            128, S, KC, 30).transpose(1, 3, 2, 0).reshape(B, T, H)
    return full
